# revision 30
# baseline (speedup 1.0000x reference)
"""Trainium2 Bass kernel for nn_ExternalInteraction.

Computation (per batch b):
    img_sum[b, d]  = sum_i image[b, i, d]
    user_sum[b, d] = sum_u user[b, u, d]
    out_user[b, u, d] = user[b, u, d] * img_sum[b, d]
    out_img[b, i, d]  = image[b, i, d] * user_sum[b, d]

Sharding: data-parallel over batch B=64 across 8 cores (8 batches/core).

Per-core kernel strategy (memory-bound: 42 MB DRAM traffic/core is the
roofline; all compute engines are kept well under it):
  - Load each batch's user [256,512] as an SBUF tile [128, 2, 512] and
    image [1024,512] as [128, 8, 512] via the contiguous "(p n) d" layout
    (each partition holds n consecutive rows -> maximally contiguous DMA).
  - VectorE tree-adds collapse each partition's row slices to a single
    per-partition partial sum [128, 512] (unit-stride operands only).
  - One fp32 TensorE matmul per tensor with an all-ones [128,128]
    stationary operand then reduces across partitions AND broadcasts the
    total to every partition of a PSUM tile [128, 512] in one shot.
  - In-place VectorE multiplies (PSUM operand broadcast along the n free
    axis with a step-0 AP), then contiguous DMA back to DRAM; the image
    multiply/store is split in halves so stores start earlier.
  - Loads issue on the SP HWDGE ring (nc.sync), stores on the ACT HWDGE
    ring (nc.scalar): stores that wait on compute would otherwise
    head-of-line block later loads queued on the same FIFO ring.
"""

import numpy as np

B, U, I, D = 64, 256, 1024, 512
N_CORES = 8
BPC = B // N_CORES  # batches per core
P = 128
NU = U // P  # 2 user row-slices per batch
NI = I // P  # 8 image row-slices per batch

_STATE = {}


def _build(repeats=1):
    from contextlib import ExitStack

    import concourse.bass as bass
    import concourse.mybir as mybir
    from concourse.tile import TileContext

    nc = bass.Bass()
    f32 = mybir.dt.float32

    user_h = nc.declare_dram_parameter("user_attributes", [BPC, U, D], f32, isOutput=False)
    img_h = nc.declare_dram_parameter("image_attributes", [BPC, I, D], f32, isOutput=False)
    ouser_h = nc.declare_dram_parameter("out_user", [BPC, U, D], f32, isOutput=True)
    oimg_h = nc.declare_dram_parameter("out_img", [BPC, I, D], f32, isOutput=True)

    with TileContext(nc) as tc, ExitStack() as ctx:
        singles = ctx.enter_context(tc.tile_pool(name="singles", bufs=1))
        in_pool = ctx.enter_context(tc.tile_pool(name="in", bufs=4))
        tmp_pool = ctx.enter_context(tc.tile_pool(name="tmp", bufs=4))
        psum_pool = ctx.enter_context(tc.tile_pool(name="psum", bufs=4, space="PSUM"))

        ones = singles.tile([P, P], f32)
        nc.vector.memset(ones, 1.0)

        for b in range(BPC * repeats):
            b = b % BPC
            usr_sb = in_pool.tile([P, NU, D], f32, tag="usr_in")
            img_sb = in_pool.tile([P, NI, D], f32, tag="img_in")
            nc.sync.dma_start(out=usr_sb, in_=user_h[b].rearrange("(p n) d -> p n d", p=P))
            nc.sync.dma_start(out=img_sb, in_=img_h[b].rearrange("(p n) d -> p n d", p=P))

            # Per-partition partial sums over each partition's row slices
            # (unit-stride tree adds on VectorE), image first so the user
            # multiply can start as early as possible.
            tmp4 = tmp_pool.tile([P, 4, D], f32, tag="tmp4")
            nc.vector.tensor_add(tmp4, img_sb[:, 0:4, :], img_sb[:, 4:8, :])
            nc.vector.tensor_add(tmp4[:, 0:2, :], tmp4[:, 0:2, :], tmp4[:, 2:4, :])
            part_i = tmp4[:, 0, :]
            nc.vector.tensor_add(part_i, part_i, tmp4[:, 1, :])
            part_u = tmp_pool.tile([P, D], f32, tag="part_u")
            nc.vector.tensor_add(part_u, usr_sb[:, 0, :], usr_sb[:, 1, :])

            # One TensorE ones-matmul per tensor finishes the reduction
            # across partitions AND broadcasts the sum to all partitions.
            isum = psum_pool.tile([P, D], f32, tag="isum")
            usum = psum_pool.tile([P, D], f32, tag="usum")
            nc.tensor.matmul(isum, ones, part_i, start=True, stop=True)
            nc.tensor.matmul(usum, ones, part_u, start=True, stop=True)

            # In-place multiplies (PSUM operand broadcast along n); user first
            # (small, unblocks its store), image in halves so each store
            # overlaps the next multiply.
            nc.vector.tensor_mul(
                usr_sb, usr_sb, isum.unsqueeze(1).broadcast_to([P, NU, D])
            )
            nc.scalar.dma_start(out=ouser_h[b].rearrange("(p n) d -> p n d", p=P), in_=usr_sb)
            oimg_ap = oimg_h[b].rearrange("(p n) d -> p n d", p=P)
            for h in range(2):
                sl = slice(h * 4, (h + 1) * 4)
                nc.vector.tensor_mul(
                    img_sb[:, sl, :],
                    img_sb[:, sl, :],
                    usum.unsqueeze(1).broadcast_to([P, 4, D]),
                )
                nc.scalar.dma_start(out=oimg_ap[:, sl, :], in_=img_sb[:, sl, :])

    _spill_waits(nc)
    return nc


# Walrus's codegen allows only a small number of sync-wait commands on some
# engine instruction encodings (fused fp32 Matmult takes just 1; TensorTensor
# only slightly more), but the Tile scheduler can attach 2-3. Moving each
# wait onto its own NoOp directly before the instruction is semantically
# identical (the engine sequencer executes them in program order) and keeps
# every compute instruction within the encoding limit.
_SPILL_CLASSES = {
    "InstMatmult",
    "InstTensorTensor",
    "InstTensorReduce",
    "InstTensorCopy",
    "InstTensorScalar",
    "InstMemSet",
    "InstActivation",
    "InstDMACopy",
    "InstDrain",
}


def _spill_waits(nc):
    import concourse.mybir as mybir

    for fn in nc.m.functions:
        for blk in fn.blocks:
            new_list = []
            changed = False
            for inst in blk.instructions:
                si = inst.sync_info
                if (
                    inst.__class__.__name__ in _SPILL_CLASSES
                    and si is not None
                    and len(si.on_wait) >= 2
                ):
                    for k, w in enumerate(si.on_wait):
                        nop = mybir.InstNoOp(
                            name=f"{inst.name}-wnop{k}",
                            engine=inst.engine,
                            ins=[],
                            outs=[],
                        )
                        nop.bass_nofuse = True
                        nop.sync_info = mybir.SyncInfo(on_wait=[w], on_update=[])
                        new_list.append(nop)
                    inst.sync_info = mybir.SyncInfo(on_wait=[], on_update=list(si.on_update))
                    changed = True
                new_list.append(inst)
            if changed:
                blk.instructions = new_list


def _get_nc():
    if "nc" not in _STATE:
        _STATE["nc"] = _build()
    return _STATE["nc"]


def kernel(user_attributes, image_attributes):
    from concourse.bass_utils import run_bass_kernel_spmd

    user = np.ascontiguousarray(np.asarray(user_attributes, dtype=np.float32))
    img = np.ascontiguousarray(np.asarray(image_attributes, dtype=np.float32))

    nc = _get_nc()
    in_maps = [
        {
            "user_attributes": user[c * BPC : (c + 1) * BPC],
            "image_attributes": img[c * BPC : (c + 1) * BPC],
        }
        for c in range(N_CORES)
    ]
    res = run_bass_kernel_spmd(nc, in_maps, list(range(N_CORES)))
    out_user = np.concatenate([res.results[c]["out_user"] for c in range(N_CORES)], axis=0)
    out_img = np.concatenate([res.results[c]["out_img"] for c in range(N_CORES)], axis=0)
    return out_user, out_img


# revision 31
# speedup vs baseline: 1.0021x; 1.0021x over previous
"""Trainium2 Bass kernel for nn_ExternalInteraction.

Computation (per batch b):
    img_sum[b, d]  = sum_i image[b, i, d]
    user_sum[b, d] = sum_u user[b, u, d]
    out_user[b, u, d] = user[b, u, d] * img_sum[b, d]
    out_img[b, i, d]  = image[b, i, d] * user_sum[b, d]

Sharding: data-parallel over batch B=64 across 8 cores (8 batches/core).

Per-core kernel strategy (memory-bound: 42 MB DRAM traffic/core is the
roofline; all compute engines are kept well under it):
  - Load each batch's user [256,512] as an SBUF tile [128, 2, 512] and
    image [1024,512] as [128, 8, 512] via the contiguous "(p n) d" layout
    (each partition holds n consecutive rows -> maximally contiguous DMA).
  - VectorE tree-adds collapse each partition's row slices to a single
    per-partition partial sum [128, 512] (unit-stride operands only).
  - One fp32 TensorE matmul per tensor with an all-ones [128,128]
    stationary operand then reduces across partitions AND broadcasts the
    total to every partition of a PSUM tile [128, 512] in one shot.
  - In-place VectorE multiplies (PSUM operand broadcast along the n free
    axis with a step-0 AP), then contiguous DMA back to DRAM; the image
    multiply/store is split in halves so stores start earlier.
  - Loads issue on the SP HWDGE ring (nc.sync), stores on the ACT HWDGE
    ring (nc.scalar): stores that wait on compute would otherwise
    head-of-line block later loads queued on the same FIFO ring.
"""

import numpy as np

B, U, I, D = 64, 256, 1024, 512
N_CORES = 8
BPC = B // N_CORES  # batches per core
P = 128
NU = U // P  # 2 user row-slices per batch
NI = I // P  # 8 image row-slices per batch

_STATE = {}


def _build(repeats=1):
    from contextlib import ExitStack

    import concourse.bass as bass
    import concourse.mybir as mybir
    from concourse.tile import TileContext

    nc = bass.Bass()
    f32 = mybir.dt.float32

    user_h = nc.declare_dram_parameter("user_attributes", [BPC, U, D], f32, isOutput=False)
    img_h = nc.declare_dram_parameter("image_attributes", [BPC, I, D], f32, isOutput=False)
    ouser_h = nc.declare_dram_parameter("out_user", [BPC, U, D], f32, isOutput=True)
    oimg_h = nc.declare_dram_parameter("out_img", [BPC, I, D], f32, isOutput=True)

    with TileContext(nc) as tc, ExitStack() as ctx:
        singles = ctx.enter_context(tc.tile_pool(name="singles", bufs=1))
        in_pool = ctx.enter_context(tc.tile_pool(name="in", bufs=4))
        tmp_pool = ctx.enter_context(tc.tile_pool(name="tmp", bufs=4))
        psum_pool = ctx.enter_context(tc.tile_pool(name="psum", bufs=4, space="PSUM"))

        ones = singles.tile([P, P], f32)
        nc.vector.memset(ones, 1.0)

        for b in range(BPC * repeats):
            b = b % BPC
            usr_sb = in_pool.tile([P, NU, D], f32, tag="usr_in")
            img_sb = in_pool.tile([P, NI, D], f32, tag="img_in")
            nc.sync.dma_start(out=usr_sb, in_=user_h[b].rearrange("(p n) d -> p n d", p=P))
            nc.sync.dma_start(out=img_sb, in_=img_h[b].rearrange("(p n) d -> p n d", p=P))

            # Per-partition partial sums over each partition's row slices
            # (unit-stride tree adds on VectorE), image first so the user
            # multiply can start as early as possible.
            tmp4 = tmp_pool.tile([P, 4, D], f32, tag="tmp4")
            nc.vector.tensor_add(tmp4, img_sb[:, 0:4, :], img_sb[:, 4:8, :])
            nc.vector.tensor_add(tmp4[:, 0:2, :], tmp4[:, 0:2, :], tmp4[:, 2:4, :])
            part_i = tmp4[:, 0, :]
            nc.vector.tensor_add(part_i, part_i, tmp4[:, 1, :])
            part_u = tmp_pool.tile([P, D], f32, tag="part_u")
            nc.vector.tensor_add(part_u, usr_sb[:, 0, :], usr_sb[:, 1, :])

            # One TensorE ones-matmul per tensor finishes the reduction
            # across partitions AND broadcasts the sum to all partitions.
            isum = psum_pool.tile([P, D], f32, tag="isum")
            usum = psum_pool.tile([P, D], f32, tag="usum")
            nc.tensor.matmul(isum, ones, part_i, start=True, stop=True)
            nc.tensor.matmul(usum, ones, part_u, start=True, stop=True)

            # In-place multiplies (PSUM operand broadcast along n); user first
            # (small, unblocks its store), image in halves so each store
            # overlaps the next multiply.
            nc.vector.tensor_mul(
                usr_sb, usr_sb, isum.unsqueeze(1).broadcast_to([P, NU, D])
            )
            nc.scalar.dma_start(out=ouser_h[b].rearrange("(p n) d -> p n d", p=P), in_=usr_sb)
            oimg_ap = oimg_h[b].rearrange("(p n) d -> p n d", p=P)
            for h in range(2):
                sl = slice(h * 4, (h + 1) * 4)
                nc.vector.tensor_mul(
                    img_sb[:, sl, :],
                    img_sb[:, sl, :],
                    usum.unsqueeze(1).broadcast_to([P, 4, D]),
                )
                nc.scalar.dma_start(out=oimg_ap[:, sl, :], in_=img_sb[:, sl, :])

    _prune_dead_const_memsets(nc)
    _spill_waits(nc)
    return nc


def _prune_dead_const_memsets(nc):
    # The bass preamble memsets a few standard constant tiles this kernel
    # never reads (the BIR verifier flags them as reader-less). They run on
    # the Pool engine BEFORE the all-engine start barrier, so every engine's
    # first real instruction waits on them. Dropping writes to never-read
    # locations is semantically free and shortens kernel startup.
    def memrefs(aps):
        return {getattr(a, "memref", None) for a in (aps or [])}

    read_refs = set()
    for fn in nc.m.functions:
        for blk in fn.blocks:
            for inst in blk.instructions:
                read_refs |= memrefs(inst.ins)
    for fn in nc.m.functions:
        for blk in fn.blocks:
            insts = list(blk.instructions)
            kept = [
                inst
                for inst in insts
                if not (
                    inst.__class__.__name__ == "InstMemset"
                    and all(
                        isinstance(r, str) and r.startswith("const-") and r not in read_refs
                        for r in memrefs(inst.outs)
                    )
                )
            ]
            if len(kept) != len(insts):
                blk.instructions = kept


# Walrus's codegen allows only a small number of sync-wait commands on some
# engine instruction encodings (fused fp32 Matmult takes just 1; TensorTensor
# only slightly more), but the Tile scheduler can attach 2-3. Moving each
# wait onto its own NoOp directly before the instruction is semantically
# identical (the engine sequencer executes them in program order) and keeps
# every compute instruction within the encoding limit.
_SPILL_CLASSES = {
    "InstMatmult",
    "InstTensorTensor",
    "InstTensorReduce",
    "InstTensorCopy",
    "InstTensorScalar",
    "InstMemSet",
    "InstActivation",
    "InstDMACopy",
    "InstDrain",
}


def _spill_waits(nc):
    import concourse.mybir as mybir

    for fn in nc.m.functions:
        for blk in fn.blocks:
            new_list = []
            changed = False
            for inst in blk.instructions:
                si = inst.sync_info
                if (
                    inst.__class__.__name__ in _SPILL_CLASSES
                    and si is not None
                    and len(si.on_wait) >= 2
                ):
                    for k, w in enumerate(si.on_wait):
                        nop = mybir.InstNoOp(
                            name=f"{inst.name}-wnop{k}",
                            engine=inst.engine,
                            ins=[],
                            outs=[],
                        )
                        nop.bass_nofuse = True
                        nop.sync_info = mybir.SyncInfo(on_wait=[w], on_update=[])
                        new_list.append(nop)
                    inst.sync_info = mybir.SyncInfo(on_wait=[], on_update=list(si.on_update))
                    changed = True
                new_list.append(inst)
            if changed:
                blk.instructions = new_list


def _get_nc():
    if "nc" not in _STATE:
        _STATE["nc"] = _build()
    return _STATE["nc"]


def kernel(user_attributes, image_attributes):
    from concourse.bass_utils import run_bass_kernel_spmd

    user = np.ascontiguousarray(np.asarray(user_attributes, dtype=np.float32))
    img = np.ascontiguousarray(np.asarray(image_attributes, dtype=np.float32))

    nc = _get_nc()
    in_maps = [
        {
            "user_attributes": user[c * BPC : (c + 1) * BPC],
            "image_attributes": img[c * BPC : (c + 1) * BPC],
        }
        for c in range(N_CORES)
    ]
    res = run_bass_kernel_spmd(nc, in_maps, list(range(N_CORES)))
    out_user = np.concatenate([res.results[c]["out_user"] for c in range(N_CORES)], axis=0)
    out_img = np.concatenate([res.results[c]["out_img"] for c in range(N_CORES)], axis=0)
    return out_user, out_img
